# revision 10
# baseline (speedup 1.0000x reference)
"""nn_Attention_3d — 3D windowed attention with decomposed relative position
biases, as a Bass/Tile kernel on 8 Trainium2 NeuronCores.

Contract: kernel(**inputs) takes FULL unsharded inputs, returns the FULL
(32, 8, 8, 8, 768) float32 output. Sharding: data-parallel over the window
dim B (4 windows per core); weights/tables replicated.

Math notes (host-side folds):
  - q' = scale*q folded into Wq/bq; rel tables scaled by lr/scale so
    attn = q'.k + q'.Rh[h_n,h_m] + q'.Rw[..] + q'.Rd[..] uniformly.
  - The three decomposed rel-pos biases are folded into the QK^T matmul via a
    24-row contraction extension: one-hot key-side rows x per-query rel values.
  - Softmax has no max-subtraction (logits are O(1) here); sums ride as a
    65th ones-column of V through the AV matmul; division is applied by a
    K=1 broadcast matmul + in-place DVE multiply, software-pipelined one
    window behind the attention pass.
"""
import threading

import numpy as np
import ml_dtypes

import concourse.bass as bass
import concourse.tile as tile
from concourse import mybir

import tile_patch  # walrus multi-wait drain workaround

tile_patch.install()

BF16 = mybir.dt.bfloat16
F32 = mybir.dt.float32
AF = mybir.ActivationFunctionType

NH = 12            # heads
HD = 64            # head dim
C = 768            # channels
N = 512            # tokens per window (8*8*8)
WPC = 4            # windows per core
NT = WPC * N       # tokens per core (2048)
NCORES = 8
SCALE = HD ** -0.5

_BF = ml_dtypes.bfloat16


def _pack_k(mat_t, cols):
    """(768, cols) -> (128, 6*cols) k-tile packed."""
    return np.ascontiguousarray(
        mat_t.reshape(6, 128, cols).transpose(1, 0, 2).reshape(128, 6 * cols)
    )


def prep_inputs(x, qkv_w, qkv_b, proj_w, proj_b, rel_pos_h, rel_pos_w,
                rel_pos_d, lr):
    x = np.asarray(x, np.float32)
    qkv_w = np.asarray(qkv_w, np.float32)
    qkv_b = np.asarray(qkv_b, np.float32)
    proj_w = np.asarray(proj_w, np.float32)
    proj_b = np.asarray(proj_b, np.float32)
    lr = float(np.asarray(lr))

    wq = qkv_w[0:C] * SCALE
    wk = qkv_w[C:2 * C]
    wv = qkv_w[2 * C:3 * C]
    bq = qkv_b[0:C] * SCALE
    bk = qkv_b[C:2 * C]
    bv = qkv_b[2 * C:3 * C]

    wqk = np.concatenate([wq, wk], 0)                     # (1536, 768)
    wqk_sb = _pack_k(wqk.T.astype(_BF), 1536)             # (128, 9216) bf16
    wv_sb = _pack_k(wv.T.astype(_BF), 768)                # (128, 4608)
    pw_sb = _pack_k(proj_w.T.astype(_BF), 768)            # (128, 4608)
    bqk_sb = np.ascontiguousarray(
        np.concatenate([bq, bk]).reshape(12, 128).T.astype(np.float32)
    )                                                     # (128, 12) f32
    brow = np.concatenate([bv, proj_b]).reshape(1, 1536).astype(_BF)

    # one-hot key-side rows: [x_T(m) == j]
    m = np.arange(N)
    planes = [m // 64, (m // 8) % 8, m % 8]
    oh = np.zeros((24, N), np.float32)
    for T in range(3):
        for j in range(8):
            oh[T * 8 + j] = (planes[T] == j)
    oh = oh.astype(_BF)

    # head-selector for the normalization broadcast matmul:
    # onesel[p, t*64 + i] = (p == t)
    onesel = np.zeros((12, 768), np.float32)
    for t in range(12):
        onesel[t, t * 64:(t + 1) * 64] = 1.0
    onesel = onesel.astype(_BF)

    # pair-blockdiag rel tables: rp[HH*64+c, T*128 + HH*64 + a*8+j] = RT[a,j,c]
    tabs = [rel_pos_h, rel_pos_w, rel_pos_d]
    rp = np.zeros((128, 384), np.float32)
    idx = np.arange(8)[:, None] - np.arange(8)[None, :] + 7   # (a, j)
    fold = lr / SCALE
    for T in range(3):
        rt = np.asarray(tabs[T], np.float32)[idx] * fold      # (8, 8, 64)
        rall = rt.transpose(2, 0, 1).reshape(64, 64)          # [c, a*8+j]
        for HH in range(2):
            rp[HH * 64:HH * 64 + 64,
               T * 128 + HH * 64:T * 128 + HH * 64 + 64] = rall
    rp = rp.astype(_BF)

    # per-core xT: (128, 6*2048)
    xs = x.reshape(NCORES, WPC, N, C)
    in_maps = []
    for core in range(NCORES):
        xt = xs[core].transpose(2, 0, 1).reshape(C, NT).astype(_BF)
        xt_sb = _pack_k(xt, NT)
        in_maps.append({
            "xT": xt_sb,
            "wqk": wqk_sb,
            "wv": wv_sb,
            "pw": pw_sb,
            "bqk": bqk_sb,
            "brow": brow,
            "oh": np.ascontiguousarray(oh),
            "rp": np.ascontiguousarray(rp),
            "onesel": np.ascontiguousarray(onesel),
        })
    return in_maps


def build_attention(tc, out_ap, ins, ctx):
    """Emit the per-core program. ins: dict name->AP (DRAM). out_ap: (2048, 768) f32."""
    nc = tc.nc

    const = ctx.enter_context(tc.tile_pool(name="const", bufs=1))
    persist = ctx.enter_context(tc.tile_pool(name="persist", bufs=1))

    # ---- small constants (live whole kernel) ----
    bqk = const.tile([128, 12], F32)
    brow = const.tile([1, 1536], BF16)
    oh = const.tile([24, 512], BF16)
    rp = const.tile([128, 384], BF16)
    onesel = const.tile([12, 768], BF16)
    ones = const.tile([1, 128], BF16)
    for t, name in [(bqk, "bqk"), (brow, "brow"), (oh, "oh"), (rp, "rp"),
                    (onesel, "onesel")]:
        nc.sync.dma_start(t[:], ins[name][:])
    nc.vector.memset(ones[:], 1.0)

    # ---- persistent intermediates ----
    qkT = persist.tile([128, 24576], BF16)    # q'/k feature-major, head-paired
    vh = persist.tile([128, 12480], BF16)     # v token-major, 65-col head groups
    o_sb = persist.tile([128, 12288], BF16)   # AV out, head-paired k-tiles
    sums = persist.tile([12, 2048], F32)
    recip_f = persist.tile([12, 2048], F32)
    recip_bf = persist.tile([12, 2048], BF16)

    vhv = vh[:].rearrange("p (tau h e) -> p tau h e", tau=16, e=65)
    nc.vector.memset(vhv[:, :, :, 64:65], 1.0)

    # ================= Phase A: QKV projections =================
    with tc.tile_pool(name="phA", bufs=1) as phA, \
         tc.tile_pool(name="psA", bufs=4, space="PSUM") as psA, \
         tc.tile_pool(name="psV", bufs=2, space="PSUM") as psV:
        xT = phA.tile([128, 12288], BF16)
        wqk = phA.tile([128, 9216], BF16)
        wv = phA.tile([128, 4608], BF16)
        nc.sync.dma_start(xT[:], ins["xT"][:])
        nc.sync.dma_start(wqk[:], ins["wqk"][:])
        nc.sync.dma_start(wv[:], ins["wv"][:])

        for j in range(12):
            for w in range(4):
                ps = psA.tile([128, 512], F32)
                for k in range(6):
                    nc.tensor.matmul(
                        ps[:],
                        wqk[:, k * 1536 + j * 128:k * 1536 + j * 128 + 128],
                        xT[:, k * 2048 + w * 512:k * 2048 + w * 512 + 512],
                        start=(k == 0), stop=(k == 5),
                    )
                dst = qkT[:, j * 2048 + w * 512:j * 2048 + w * 512 + 512]
                if (j * 4 + w) % 2 == 0:
                    nc.scalar.activation(dst, ps[:], AF.Identity,
                                         bias=bqk[:, j:j + 1])
                else:
                    nc.vector.tensor_scalar_add(dst, ps[:], bqk[:, j:j + 1])

        for tau in range(16):
            psv = psV.tile([128, 1024], F32)
            lhs = [xT[:, k * 2048 + tau * 128:k * 2048 + tau * 128 + 128]
                   for k in range(6)]
            for k in range(6):
                nc.tensor.matmul(psv[:, 0:512], lhs[k],
                                 wv[:, k * 768:k * 768 + 512],
                                 start=(k == 0), stop=False)
            nc.tensor.matmul(psv[:, 0:512], ones[0:1, 0:128],
                             brow[0:1, 0:512], start=False, stop=True)
            for k in range(6):
                nc.tensor.matmul(psv[:, 512:768], lhs[k],
                                 wv[:, k * 768 + 512:k * 768 + 768],
                                 start=(k == 0), stop=False)
            nc.tensor.matmul(psv[:, 512:768], ones[0:1, 0:128],
                             brow[0:1, 512:768], start=False, stop=True)
            dst = vhv[:, tau, :, 0:64]
            src = psv[:, 0:768].rearrange("p (h e) -> p h e", e=64)
            if tau % 2 == 0:
                nc.vector.tensor_copy(dst, src)
            else:
                nc.scalar.copy(dst, src)

    with tc.tile_pool(name="phBC", bufs=1) as phBC:
        rel = phBC.tile([24, 24576], BF16)   # (T*8+j, t*2048 + w*512 + n)

        # ================= Phase B: rel-pos values =================
        with tc.tile_pool(name="psB", bufs=2, space="PSUM") as psB, \
             tc.tile_pool(name="stgB", bufs=3) as stgB:
            for j6 in range(6):
                for T in range(3):
                    ps = psB.tile([128, 2048], F32)
                    for w in range(4):
                        nc.tensor.matmul(
                            ps[:, w * 512:w * 512 + 512],
                            rp[:, T * 128:T * 128 + 128],
                            qkT[:, j6 * 2048 + w * 512:j6 * 2048 + w * 512 + 512],
                            start=True, stop=True,
                        )
                    stg = stgB.tile([128, 2048], BF16)
                    if (j6 * 3 + T) % 2 == 0:
                        nc.vector.tensor_copy(stg[:], ps[:])
                    else:
                        nc.scalar.copy(stg[:], ps[:])
                    # scatter: rel[8T+j, (2*j6+H)*2048 + w*512 + n] =
                    #          stg[H*64 + xT(n)*8 + j, w*512 + n]
                    sv = stg[:].rearrange("(H A j) (w n) -> H A j w n", H=2,
                                          A=8, n=512)
                    dv = rel[8 * T:8 * T + 8, :] \
                        .rearrange("j (t w n) -> j t w n", w=4, n=512) \
                        [:, 2 * j6:2 * j6 + 2] \
                        .transpose([1, 0, 2, 3])          # (H, j, w, n)
                    # DMA balancer caps APs at 3 dims -> loop H (and w
                    # for the middle table) explicitly.
                    for a in range(8):
                        for H in range(2):
                            s_a = sv[H, a]                 # (j, w, n)
                            d_a = dv[H]                    # (j, w, n)
                            if T == 0:
                                s = s_a[:, :, a * 64:a * 64 + 64]
                                d = d_a[:, :, a * 64:a * 64 + 64]
                                nc.sync.dma_start(d, s)
                            elif T == 1:
                                sr = s_a.rearrange(
                                    "j w (h ww d) -> j w h ww d", ww=8, d=8)
                                dr = d_a.rearrange(
                                    "j w (h ww d) -> j w h ww d", ww=8, d=8)
                                for w in range(4):
                                    nc.sync.dma_start(dr[:, w, :, a, :],
                                                      sr[:, w, :, a, :])
                            else:
                                s = s_a.rearrange("j w (hw d) -> j w hw d",
                                                  d=8)[:, :, :, a]
                                d = d_a.rearrange("j w (hw d) -> j w hw d",
                                                  d=8)[:, :, :, a]
                                nc.sync.dma_start(d, s)

        # ============ Phase C: attention per (window, head) ============
        with tc.tile_pool(name="psC", bufs=2, space="PSUM") as psC, \
             tc.tile_pool(name="psAV", bufs=1, space="PSUM") as psAV, \
             tc.tile_pool(name="psBC", bufs=2, space="PSUM") as psBC, \
             tc.tile_pool(name="expp", bufs=2) as expp, \
             tc.tile_pool(name="srowp", bufs=3) as srowp:

            def norm_pass(w, t):
                """Divide head t / window w AV output by its softmax sums."""
                H = t % 2
                qcol = (t // 2) * 2048 + w * 512
                psbc = psBC.tile([64, 512], F32)
                nc.tensor.matmul(
                    psbc[:], onesel[0:12, t * 64:t * 64 + 64],
                    recip_bf[0:12, w * 512:w * 512 + 512],
                    start=True, stop=True)
                dst = o_sb[64 * H:64 * H + 64, qcol:qcol + 512]
                nc.vector.tensor_mul(dst, dst, psbc[:])

            for w in range(4):
                for t in range(12):
                    H = t % 2
                    qcol = (t // 2) * 2048 + w * 512
                    kcol = (6 + t // 2) * 2048 + w * 512
                    exp_sb = expp.tile([128, 2048], BF16)
                    for hc in range(2):
                        ps = psC.tile([128, 1024], F32)
                        for ci in range(2):
                            c = 2 * hc + ci
                            dst = ps[:, ci * 512:ci * 512 + 512]
                            nc.tensor.matmul(
                                dst,
                                qkT[64 * H:64 * H + 64,
                                    kcol + c * 128:kcol + c * 128 + 128],
                                qkT[64 * H:64 * H + 64, qcol:qcol + 512],
                                start=True, stop=False,
                            )
                            nc.tensor.matmul(
                                dst,
                                oh[:, c * 128:c * 128 + 128],
                                rel[:, t * 2048 + w * 512:
                                    t * 2048 + w * 512 + 512],
                                start=False, stop=True,
                            )
                        nc.scalar.activation(
                            exp_sb[:, hc * 1024:hc * 1024 + 1024], ps[:],
                            AF.Exp)

                    psav = psAV.tile([65, 512], F32)
                    for c in range(4):
                        nc.tensor.matmul(
                            psav[:],
                            vh[:, (w * 4 + c) * 780 + t * 65:
                               (w * 4 + c) * 780 + t * 65 + 65],
                            exp_sb[:, c * 512:c * 512 + 512],
                            start=(c == 0), stop=(c == 3),
                        )
                    # engines need 32-aligned start partitions: bounce the
                    # sums row via a base-0 staging row, then DMA to row t.
                    srow = srowp.tile([1, 512], F32)
                    nc.scalar.copy(srow[:], psav[64:65, :])
                    nc.sync.dma_start(sums[t:t + 1, w * 512:w * 512 + 512],
                                      srow[:])
                    # stash unnormalized AV output (normalized later in-place)
                    nc.vector.tensor_copy(
                        o_sb[64 * H:64 * H + 64, qcol:qcol + 512],
                        psav[0:64, :])

                    # software-pipelined normalization of the previous window
                    if w > 0:
                        norm_pass(w - 1, t)

                # reciprocal of softmax sums for window w
                nc.vector.reciprocal(
                    recip_f[:, w * 512:w * 512 + 512],
                    sums[:, w * 512:w * 512 + 512])
                nc.vector.tensor_copy(recip_bf[:, w * 512:w * 512 + 512],
                                      recip_f[:, w * 512:w * 512 + 512])

            for t in range(12):
                norm_pass(3, t)

    # ================= Phase D: output projection =================
    with tc.tile_pool(name="phD", bufs=1) as phD, \
         tc.tile_pool(name="psD", bufs=4, space="PSUM") as psD, \
         tc.tile_pool(name="outp", bufs=3) as outp:
        pw = phD.tile([128, 4608], BF16)
        nc.sync.dma_start(pw[:], ins["pw"][:])
        for tau in range(16):
            psd = psD.tile([128, 1024], F32)
            lhs = [o_sb[:, k * 2048 + tau * 128:k * 2048 + tau * 128 + 128]
                   for k in range(6)]
            for k in range(6):
                nc.tensor.matmul(psd[:, 0:512], lhs[k],
                                 pw[:, k * 768:k * 768 + 512],
                                 start=(k == 0), stop=False)
            nc.tensor.matmul(psd[:, 0:512], ones[0:1, 0:128],
                             brow[0:1, 768:1280], start=False, stop=True)
            for k in range(6):
                nc.tensor.matmul(psd[:, 512:768], lhs[k],
                                 pw[:, k * 768 + 512:k * 768 + 768],
                                 start=(k == 0), stop=False)
            nc.tensor.matmul(psd[:, 512:768], ones[0:1, 0:128],
                             brow[0:1, 1280:1536], start=False, stop=True)
            ot = outp.tile([128, 768], F32)
            if tau % 2 == 0:
                nc.vector.tensor_copy(ot[:], psd[:, 0:768])
            else:
                nc.scalar.copy(ot[:], psd[:, 0:768])
            nc.sync.dma_start(out_ap[tau * 128:tau * 128 + 128, :], ot[:])


_BUILD_LOCK = threading.Lock()
_NC_CACHE = {}


def _build_nc():
    if "nc" in _NC_CACHE:
        return _NC_CACHE["nc"]
    from contextlib import ExitStack
    nc = bass.Bass()
    ins = {
        "xT": nc.dram_tensor("xT", [128, 12288], BF16, kind="ExternalInput"),
        "wqk": nc.dram_tensor("wqk", [128, 9216], BF16, kind="ExternalInput"),
        "wv": nc.dram_tensor("wv", [128, 4608], BF16, kind="ExternalInput"),
        "pw": nc.dram_tensor("pw", [128, 4608], BF16, kind="ExternalInput"),
        "bqk": nc.dram_tensor("bqk", [128, 12], F32, kind="ExternalInput"),
        "brow": nc.dram_tensor("brow", [1, 1536], BF16, kind="ExternalInput"),
        "oh": nc.dram_tensor("oh", [24, 512], BF16, kind="ExternalInput"),
        "rp": nc.dram_tensor("rp", [128, 384], BF16, kind="ExternalInput"),
        "onesel": nc.dram_tensor("onesel", [12, 768], BF16,
                                 kind="ExternalInput"),
    }
    out = nc.dram_tensor("out", [NT, C], F32, kind="ExternalOutput")
    with tile.TileContext(nc) as tc:
        with ExitStack() as ctx:
            build_attention(tc, out, ins, ctx)
    tile_patch.split_excess_waits(nc)
    _NC_CACHE["nc"] = nc
    return nc


def kernel(x, qkv_w, qkv_b, proj_w, proj_b, rel_pos_h, rel_pos_w, rel_pos_d,
           lr):
    from concourse.bass_utils import run_bass_kernel_spmd

    in_maps = prep_inputs(x, qkv_w, qkv_b, proj_w, proj_b, rel_pos_h,
                          rel_pos_w, rel_pos_d, lr)
    with _BUILD_LOCK:
        nc = _build_nc()
    res = run_bass_kernel_spmd(nc, in_maps, list(range(NCORES)))
    out = np.empty((NCORES, WPC, 8, 8, 8, C), np.float32)
    for core in range(NCORES):
        out[core] = res.results[core]["out"].reshape(WPC, 8, 8, 8, C)
    return out.reshape(NCORES * WPC, 8, 8, 8, C)
